# revision 13
# baseline (speedup 1.0000x reference)
"""HGCN encoder forward on 8 Trainium2 NeuronCores.

Computation (per batch b):
    w_abs = |gelu(states @ W1.T + b1) @ W2.T + b2|          (E,)  [host, tiny]
    d[n]    = sum_e H[n,e] * w_abs[e]                        (N,)
    dinv[n] = rsqrt(d[n])  (d > 0 always for these inputs)
    X[e,dd] = leaky_relu( sum_n (H[n,e]*w_abs[e]) * (dinv[n]*nf[n,dd]) )

Sharding: core c -> (batch b = c//2, node-half c%2) so each core owns
4096 full node rows (H slice 32 MiB). Per 128-node tile, one fused DVE
tensor_tensor_reduce produces both Hw = H*w_abs (matmul rhs) and the
row-reduction d. PE accumulates s.T @ Hw into 4 PSUM banks across all
32 tiles. Host sums the two per-batch partials and applies leaky_relu.
"""

import sys

for _p in ("/opt/trn_rl_repo",):
    if _p not in sys.path:
        sys.path.insert(0, _p)

import numpy as np

B, N, E, S, D = 4, 8192, 2048, 64, 16
NCORES = 8
NSHARD = N // 2          # nodes per core
NT = NSHARD // 128       # 32 tiles per core
ECH = 512                # e-chunk per matmul (one PSUM bank, fp32 max)
NJ = E // ECH            # 4 matmuls per tile

_CACHE = {}


def _build_nc():
    import concourse.bass as bass  # noqa: F401
    import concourse.mybir as mybir
    import concourse.tile as tile
    from concourse import bacc

    f32 = mybir.dt.float32
    nc = bacc.Bacc(
        "TRN2",
        target_bir_lowering=False,
        debug=False,
        num_devices=NCORES,
    )
    hg = nc.dram_tensor("hg", [NT, 128, E], f32, kind="ExternalInput").ap()
    nf = nc.dram_tensor("nf", [NT, 128, D], f32, kind="ExternalInput").ap()
    wb = nc.dram_tensor("wb", [128, E], f32, kind="ExternalInput").ap()
    y = nc.dram_tensor("y", [D, E], f32, kind="ExternalOutput").ap()

    with tile.TileContext(nc) as tc:
        with (
            tc.tile_pool(name="hpool", bufs=6) as hpool,
            tc.tile_pool(name="hwpool", bufs=5) as hwpool,
            tc.tile_pool(name="wpool", bufs=1) as wpool,
            tc.tile_pool(name="small", bufs=6) as small,
            tc.tile_pool(name="ypool", bufs=1) as ypool,
            tc.tile_pool(name="psum", bufs=1, space="PSUM") as psum_pool,
        ):
            w_tile = wpool.tile([128, E], f32, tag="w")
            nc.sync.dma_start(w_tile[:], wb[:])

            # [D, 512] accumulators, one PSUM bank per e-chunk. lhsT = s
            # (cheap 16-col weight load), hw streams as the moving operand.
            # Interleaved accumulation groups are safe across DIFFERENT
            # banks (same-bank interleaving corrupts results on HW, and
            # per-chunk self-loading fp32 weights cost ~220ns/matmul).
            accs = [
                psum_pool.tile([D, ECH], f32, tag=f"acc{j}", name=f"acc{j}")
                for j in range(NJ)
            ]

            for i in range(NT):
                h_tile = hpool.tile([128, E], f32, tag="h")
                nc.sync.dma_start(h_tile[:, : E // 2], hg[i][:, : E // 2])
                nc.sync.dma_start(h_tile[:, E // 2 :], hg[i][:, E // 2 :])
                nf_tile = small.tile([128, D], f32, tag="nf")
                nc.sync.dma_start(nf_tile[:], nf[i])

                hw_tile = hwpool.tile([128, E], f32, tag="hw")
                d_t = small.tile([128, 1], f32, tag="d")
                # hw = (H * 1.0) * w_abs ; d = sum_e hw   (single DVE pass)
                nc.vector.scalar_tensor_tensor(
                    out=hw_tile[:],
                    in0=h_tile[:],
                    scalar=1.0,
                    in1=w_tile[:],
                    op0=mybir.AluOpType.mult,
                    op1=mybir.AluOpType.mult,
                    accum_out=d_t[:],
                )
                sq_t = small.tile([128, 1], f32, tag="sq")
                nc.scalar.sqrt(sq_t[:], d_t[:])
                dinv_t = small.tile([128, 1], f32, tag="dinv")
                nc.vector.reciprocal(dinv_t[:], sq_t[:])
                s_tile = small.tile([128, D], f32, tag="s")
                nc.scalar.mul(s_tile[:], nf_tile[:], dinv_t[:])

                for j in range(NJ):
                    nc.tensor.matmul(
                        accs[j][:],
                        lhsT=s_tile[:],
                        rhs=hw_tile[:, j * ECH : (j + 1) * ECH],
                        start=(i == 0),
                        stop=(i == NT - 1),
                    )

            y_tile = ypool.tile([D, E], f32, tag="y")
            for j in range(NJ):
                nc.scalar.copy(y_tile[:, j * ECH : (j + 1) * ECH], accs[j][:])
            nc.sync.dma_start(y[:], y_tile[:])

    nc.compile()
    return nc


def _get_nc():
    if "nc" not in _CACHE:
        _CACHE["nc"] = _build_nc()
    return _CACHE["nc"]


def _host_wabs(states, W1, b1, W2, b2):
    from scipy.special import erf

    st = states.astype(np.float64)
    h = st @ W1.astype(np.float64).T + b1.astype(np.float64)
    h = h * 0.5 * (1.0 + erf(h / np.sqrt(2.0)))
    w = h @ W2.astype(np.float64).T + b2.astype(np.float64)
    return np.abs(w).astype(np.float32)  # (B, E)


def _make_in_maps(node_features, hyper_graph, w_abs):
    in_maps = []
    for c in range(NCORES):
        b, half = c // 2, c % 2
        sl = slice(half * NSHARD, (half + 1) * NSHARD)
        hg_c = np.ascontiguousarray(hyper_graph[b, sl]).reshape(NT, 128, E)
        nf_c = np.ascontiguousarray(node_features[b, sl]).reshape(NT, 128, D)
        wb_c = np.ascontiguousarray(
            np.broadcast_to(w_abs[b][None, :], (128, E))
        )
        in_maps.append({"hg": hg_c, "nf": nf_c, "wb": wb_c})
    return in_maps


def kernel(**inputs):
    from concourse.bass_utils import run_bass_kernel_spmd

    node_features = np.asarray(inputs["node_features"], dtype=np.float32)
    hyper_graph = np.asarray(inputs["hyper_graph"], dtype=np.float32)
    states = np.asarray(inputs["states"], dtype=np.float32)
    W1 = np.asarray(inputs["W1"], dtype=np.float32)
    b1 = np.asarray(inputs["b1"], dtype=np.float32)
    W2 = np.asarray(inputs["W2"], dtype=np.float32)
    b2 = np.asarray(inputs["b2"], dtype=np.float32)

    w_abs = _host_wabs(states, W1, b1, W2, b2)
    in_maps = _make_in_maps(node_features, hyper_graph, w_abs)

    nc = _get_nc()
    res = run_bass_kernel_spmd(nc, in_maps, core_ids=list(range(NCORES)))

    X = np.empty((B, E, D), dtype=np.float32)
    for b in range(B):
        p = res.results[2 * b]["y"] + res.results[2 * b + 1]["y"]  # (D, E)
        xb = p.T
        X[b] = np.where(xb >= 0, xb, np.float32(0.1) * xb)
    return X
